# revision 25
# baseline (speedup 1.0000x reference)
"""AdaLoRA routed-LoRA kernel for 8 Trainium2 NeuronCores.

Problem (nn_AdaLoRA): per token t with expert index i:
    ds[t, :]  = slots[t, :] @ down_table[i]            # [1024] @ [1024, 16]
    out[t, :] = (ds[t, :] @ up_table[i]) / sqrt(16)    # [16] @ [16, 1024]

Sharding: data-parallel over batch (B=8 -> one batch row per core; LoRA
tables replicated on every core). Per core: 256 tokens = 2 tiles of 128
tokens (tokens on SBUF partitions). ~32MB of table gather per core; the
kernel targets the DMA roofline with compute hidden under the gather.

Down projection (DVE): indirect-DMA gather each token's 64KB down row
into its partition (two 32KB chunks), then per rank r a fused
scalar_tensor_tensor(mult, mult) with accum_out reduces
slots[t,:]*down_i[:,r] in one pass.

Up projection (TensorEngine): tokens are processed in groups of 8; for
group g a [128,128] @ [128,1024] matmul contracts k=(j,r) against a
block-diagonal lhsT holding ds values (built on-chip from ds via
TensorE transpose + a replicate matmul + affine_select masks), with
rhs = the 8 tokens' up tables gathered as 16 rows each via
host-precomputed indices idx*16+r. All 16 group matmuls accumulate into
one PSUM tile (wrong-token columns are zero). f16 matmul inputs, f32
PSUM accumulation. The 1/sqrt(16) scale folds into the PSUM->SBUF copy
on the scalar engine.
"""

import numpy as np

B, K, DIM, RANK, NE = 8, 256, 1024, 16, 4096
ROW = DIM * RANK  # 16384 elements per down-table row
SCALE = 1.0 / 4.0  # 1/sqrt(RANK)
P = 128
N_TILE = K // P  # 2 token tiles per core
DCH = 2  # down-table chunks per tile (8 ranks each)
RSLOT = 4  # ranks per partition in the up gather (16KB descriptors)
TPG = P // RSLOT  # 32 tokens per up group
NGRP = P // TPG  # 4 up groups per tile
N_CORES = 8

_CACHE = {}


def _build():
    from concourse import bacc, bass, mybir, tile

    f32 = mybir.dt.float32
    f16 = mybir.dt.float16
    bf16 = mybir.dt.bfloat16
    i32 = mybir.dt.int32
    mult = mybir.AluOpType.mult
    add = mybir.AluOpType.add
    is_equal = mybir.AluOpType.is_equal

    nc = bacc.Bacc("TRN2", target_bir_lowering=False)
    slots = nc.declare_dram_parameter("slots", [K, DIM], f16, isOutput=False)
    idx = nc.declare_dram_parameter("idx", [K, 1], i32, isOutput=False)
    idx4 = nc.declare_dram_parameter("idx4", [K * RSLOT, 1], i32, isOutput=False)
    down = nc.declare_dram_parameter("down", [NE, ROW], f16, isOutput=False)
    up4 = nc.declare_dram_parameter("up4", [NE * RSLOT, RSLOT * DIM], f16, isOutput=False)
    out = nc.declare_dram_parameter("out", [K, DIM], f32, isOutput=True)


    with tile.TileContext(nc) as tc:
        with (
            tc.tile_pool(name="io", bufs=2) as io_pool,
            tc.tile_pool(name="gather", bufs=2) as gpool,
            tc.tile_pool(name="upg", bufs=8) as upool,
            tc.tile_pool(name="misc", bufs=1) as mpool,
            tc.tile_pool(name="ps", bufs=2, space="PSUM") as pspool,
            tc.tile_pool(name="psout", bufs=2, space="PSUM") as pspool_out,
        ):
            scratch = mpool.tile([P, DIM], f16)
            RC = RANK // DCH  # ranks per down chunk

            # ---- index/slot DMAs first (the first gather gates everything) ----
            idx_tiles, idx4_tiles, slots_tiles = [], [], []
            for t in range(N_TILE):
                tok = slice(t * P, (t + 1) * P)
                idx_t = io_pool.tile([P, 1], i32, tag="idx")
                nc.sync.dma_start(out=idx_t[:], in_=idx[tok, :])
                idx_tiles.append(idx_t)
                idx4_t = io_pool.tile([P, NGRP], i32, tag="idx4")
                nc.sync.dma_start(
                    out=idx4_t[:],
                    in_=idx4[t * P * RSLOT : (t + 1) * P * RSLOT, 0].rearrange(
                        "(p g) -> p g", g=NGRP
                    ),
                )
                idx4_tiles.append(idx4_t)
                slots16 = io_pool.tile([P, DIM], f16, tag="slots16")
                nc.sync.dma_start(out=slots16[:], in_=slots[tok, :])
                slots_tiles.append(slots16)

            # ---- host-precomputed constants (needed ~halfway in) ----
            ident = mpool.tile([P, P], f16)
            nc.sync.dma_start(out=ident[:], in_=ident_c[:])
            E_pack = mpool.tile([RANK, RSLOT * P], f16)
            nc.sync.dma_start(out=E_pack[:], in_=e_c[:])
            M4 = mpool.tile([P, P], f16)  # M4[p, t] = (p//4 == t%32)
            nc.sync.dma_start(out=M4[:], in_=m4_c[:])
            # zero-padded lhsT buffers, (tile, h, g): zero except columns
            # 32g..32g+32 (refilled per tile; zeros persist)
            lhsT_all = mpool.tile([P, N_TILE, RSLOT, NGRP, P], f16)
            nc.scalar.memzero(lhsT_all[:])

            # ---- phase A: down gathers + down projection + lhsT build ----
            for t in range(N_TILE):
                idx_t = idx_tiles[t]
                slots16 = slots_tiles[t]
                # down projection -> ds16 [128, 16] f16. The very first
                # chunk is split small so the DVE stream starts early.
                chunk_plan = [4, 4, 8] if t == 0 else [8, 8]
                ds_a = io_pool.tile([P, RANK // 2], f32, tag="ds_a")  # even ranks (ACT)
                ds_b = io_pool.tile([P, RANK // 2], f32, tag="ds_b")  # odd ranks (DVE)
                r0 = 0
                for nr in chunk_plan:
                    dch = gpool.tile([P, nr, DIM], f16, tag=f"dch{nr}")
                    nc.gpsimd.indirect_dma_start(
                        out=dch[:].rearrange("p r d -> p (r d)"),
                        out_offset=None,
                        in_=down[:],
                        in_offset=bass.IndirectOffsetOnAxis(ap=idx_t[:, :1], axis=0),
                        element_offset=r0 * DIM,
                    )
                    for rl in range(nr):
                        r = r0 + rl
                        if rl % 2 == 0:
                            # DVE 4x product + ACT free-dim sum
                            prod = upool.tile([P, DIM], f16, tag="prod")
                            nc.vector.tensor_tensor(
                                out=prod[:],
                                in0=slots16[:],
                                in1=dch[:, rl, :],
                                op=mult,
                            )
                            nc.scalar.activation(
                                out=scratch[:],
                                in_=prod[:],
                                func=mybir.ActivationFunctionType.Copy,
                                accum_out=ds_a[:, r // 2 : r // 2 + 1],
                            )
                        else:
                            # fused multiply+reduce on DVE
                            nc.vector.scalar_tensor_tensor(
                                out=scratch[:],
                                in0=slots16[:],
                                scalar=1.0,
                                in1=dch[:, rl, :],
                                op0=mult,
                                op1=mult,
                                accum_out=ds_b[:, r // 2 : r // 2 + 1],
                            )
                    r0 += nr

                ds16 = io_pool.tile([P, RANK], f16, tag="ds16")
                nc.vector.tensor_copy(
                    out=ds16[:].rearrange("p (a two) -> p a two", two=2)[:, :, 0],
                    in_=ds_a[:],
                )
                nc.vector.tensor_copy(
                    out=ds16[:].rearrange("p (a two) -> p a two", two=2)[:, :, 1],
                    in_=ds_b[:],
                )
                # build the block-diagonal lhsT family from ds
                dsT_psum = pspool.tile([RANK, P], f16, space="PSUM", tag="dsT")
                nc.tensor.transpose(out=dsT_psum[:], in_=ds16[:], identity=ident[:])
                dsT = io_pool.tile([RANK, P], f16, tag="dsT")
                nc.vector.tensor_copy(out=dsT[:], in_=dsT_psum[:])
                for h in range(RSLOT):
                    rep_psum = pspool.tile([P, P], f32, space="PSUM", tag="rep")
                    nc.tensor.matmul(
                        out=rep_psum[:],
                        lhsT=E_pack[:, h * P : (h + 1) * P],
                        rhs=dsT[:],
                        start=True,
                        stop=True,
                    )
                    for g in range(NGRP):
                        cs = slice(TPG * g, TPG * (g + 1))
                        nc.vector.tensor_tensor(
                            out=lhsT_all[:, t, h, g, cs],
                            in0=rep_psum[:, cs],
                            in1=M4[:, cs],
                            op=mult,
                        )

            # ---- phase B: up projection on TensorE ----
            for t in range(N_TILE):
                tok = slice(t * P, (t + 1) * P)
                out_psum = pspool_out.tile([P, DIM], f32, space="PSUM", tag="outp")
                for g in range(NGRP):
                    upc = upool.tile([P, RSLOT * DIM], f16, tag="upc")
                    nc.gpsimd.indirect_dma_start(
                        out=upc[:],
                        out_offset=None,
                        in_=up4[:],
                        in_offset=bass.IndirectOffsetOnAxis(
                            ap=idx4_tiles[t][:, g : g + 1], axis=0
                        ),
                    )
                    for h in range(RSLOT):
                        for n in range(2):
                            n0, n1 = n * 512, (n + 1) * 512
                            nc.tensor.matmul(
                                out=out_psum[:, n0:n1],
                                lhsT=lhsT_all[:, t, h, g, :],
                                rhs=upc[:, h * DIM + n0 : h * DIM + n1],
                                start=(g == 0 and h == 0),
                                stop=(g == NGRP - 1 and h == RSLOT - 1),
                            )
                out_sb = io_pool.tile([P, DIM], f32, tag="osb")
                nc.scalar.mul(out_sb[:], out_psum[:], SCALE)
                nc.sync.dma_start(out=out[tok, :], in_=out_sb[:])
    nc.compile()
    return nc


def _get_nc():
    if "nc" not in _CACHE:
        _CACHE["nc"] = _build()
    return _CACHE["nc"]


def _prep_in_maps(slots, indices, down_proj_values, up_proj_values):
    slots = np.ascontiguousarray(np.asarray(slots, dtype=np.float32).astype(np.float16))
    indices = np.ascontiguousarray(np.asarray(indices).astype(np.int32))
    down = np.ascontiguousarray(
        np.asarray(down_proj_values, dtype=np.float32)
        .transpose(0, 2, 1)
        .reshape(NE, ROW)
        .astype(np.float16)
    )
    up4 = np.ascontiguousarray(
        np.asarray(up_proj_values, dtype=np.float32)
        .reshape(NE * RSLOT, RSLOT * DIM)
        .astype(np.float16)
    )
    assert slots.shape == (B, K, DIM) and indices.shape == (B, K)
    # idx4[t*512 + g*128 + p] = indices[128*t + 32*g + p//4]*4 + p%4
    # (tile t, group g of 32 tokens; partition p = (j, rp) = (p//4, p%4);
    #  up4 table viewed as [NE*4, 4096]: row idx*4+rp = ranks 4rp..4rp+4)
    p = np.arange(P)
    j, rp = p // RSLOT, p % RSLOT
    t_i = np.arange(N_TILE)[:, None, None]
    g_i = np.arange(NGRP)[None, :, None]
    toks = 128 * t_i + TPG * g_i + j[None, None, :]  # [N_TILE, NGRP, P]
    # host-built constants
    ident_c = np.eye(P, dtype=np.float16)
    e_c = np.zeros((RANK, RSLOT, P), np.float16)  # E_h[q, x] = (q == 4*(x%4)+h)
    for h in range(RSLOT):
        for x in range(P):
            e_c[RSLOT * (x % RSLOT) + h, h, x] = 1.0
    e_c = e_c.transpose(0, 1, 2).reshape(RANK, RSLOT * P)
    m4_c = (np.arange(P)[:, None] // RSLOT == np.arange(P)[None, :] % TPG).astype(
        np.float16
    )
    in_maps = []
    for i in range(N_CORES):
        idx4v = (
            (indices[i][toks] * RSLOT + rp[None, None, :])
            .astype(np.int32)
            .transpose(0, 2, 1)
        )  # [N_TILE, P, NGRP]
        in_maps.append(
            {
                "slots": slots[i],
                "idx": indices[i].reshape(K, 1),
                "idx4": idx4v.reshape(K * RSLOT, 1),
                "down": down,
                "up4": up4,
                "ident_c": ident_c,
                "e_c": e_c,
                "m4_c": m4_c,
            }
        )
    return in_maps


def _run(in_maps, trace=False):
    from concourse.bass_utils import run_bass_kernel_spmd

    nc = _get_nc()
    return run_bass_kernel_spmd(
        nc, in_maps, core_ids=list(range(N_CORES)), trace=trace
    )


def kernel(slots, indices, down_proj_values, up_proj_values):
    in_maps = _prep_in_maps(slots, indices, down_proj_values, up_proj_values)
    res = _run(in_maps)
    out = np.stack([res.results[i]["out"] for i in range(N_CORES)], axis=0)
    return out.astype(np.float32)


# revision 26
# speedup vs baseline: 1.0368x; 1.0368x over previous
"""AdaLoRA routed-LoRA kernel for 8 Trainium2 NeuronCores.

Problem (nn_AdaLoRA): per token t with expert index i:
    ds[t, :]  = slots[t, :] @ down_table[i]            # [1024] @ [1024, 16]
    out[t, :] = (ds[t, :] @ up_table[i]) / sqrt(16)    # [16] @ [16, 1024]

Sharding: data-parallel over batch (B=8 -> one batch row per core; LoRA
tables replicated on every core). Per core: 256 tokens = 2 tiles of 128
tokens (tokens on SBUF partitions). ~32MB of table gather per core; the
kernel targets the DMA roofline with compute hidden under the gather.

Down projection (DVE): indirect-DMA gather each token's 64KB down row
into its partition (two 32KB chunks), then per rank r a fused
scalar_tensor_tensor(mult, mult) with accum_out reduces
slots[t,:]*down_i[:,r] in one pass.

Up projection (TensorEngine): tokens are processed in groups of 8; for
group g a [128,128] @ [128,1024] matmul contracts k=(j,r) against a
block-diagonal lhsT holding ds values (built on-chip from ds via
TensorE transpose + a replicate matmul + affine_select masks), with
rhs = the 8 tokens' up tables gathered as 16 rows each via
host-precomputed indices idx*16+r. All 16 group matmuls accumulate into
one PSUM tile (wrong-token columns are zero). f16 matmul inputs, f32
PSUM accumulation. The 1/sqrt(16) scale folds into the PSUM->SBUF copy
on the scalar engine.
"""

import numpy as np

B, K, DIM, RANK, NE = 8, 256, 1024, 16, 4096
ROW = DIM * RANK  # 16384 elements per down-table row
SCALE = 1.0 / 4.0  # 1/sqrt(RANK)
P = 128
N_TILE = K // P  # 2 token tiles per core
DCH = 2  # down-table chunks per tile (8 ranks each)
RSLOT = 4  # ranks per partition in the up gather (16KB descriptors)
TPG = P // RSLOT  # 32 tokens per up group
NGRP = P // TPG  # 4 up groups per tile
N_CORES = 8

_CACHE = {}


def _build():
    from concourse import bacc, bass, mybir, tile

    f32 = mybir.dt.float32
    f16 = mybir.dt.float16
    bf16 = mybir.dt.bfloat16
    i32 = mybir.dt.int32
    mult = mybir.AluOpType.mult
    add = mybir.AluOpType.add
    is_equal = mybir.AluOpType.is_equal

    nc = bacc.Bacc("TRN2", target_bir_lowering=False)
    slots = nc.declare_dram_parameter("slots", [K, DIM], f16, isOutput=False)
    idx = nc.declare_dram_parameter("idx", [K, 1], i32, isOutput=False)
    idx4 = nc.declare_dram_parameter("idx4", [K * RSLOT, 1], i32, isOutput=False)
    down = nc.declare_dram_parameter("down", [NE, ROW], f16, isOutput=False)
    up4 = nc.declare_dram_parameter("up4", [NE * RSLOT, RSLOT * DIM], f16, isOutput=False)
    out = nc.declare_dram_parameter("out", [K, DIM], f32, isOutput=True)


    with tile.TileContext(nc) as tc:
        with (
            tc.tile_pool(name="io", bufs=2) as io_pool,
            tc.tile_pool(name="gather", bufs=2) as gpool,
            tc.tile_pool(name="upg", bufs=8) as upool,
            tc.tile_pool(name="misc", bufs=1) as mpool,
            tc.tile_pool(name="ps", bufs=2, space="PSUM") as pspool,
            tc.tile_pool(name="psout", bufs=2, space="PSUM") as pspool_out,
        ):
            scratch = mpool.tile([P, DIM], f16)
            RC = RANK // DCH  # ranks per down chunk

            # ---- index/slot DMAs first (the first gather gates everything) ----
            idx_tiles, idx4_tiles, slots_tiles = [], [], []
            for t in range(N_TILE):
                tok = slice(t * P, (t + 1) * P)
                idx_t = io_pool.tile([P, 1], i32, tag="idx")
                nc.sync.dma_start(out=idx_t[:], in_=idx[tok, :])
                idx_tiles.append(idx_t)
                idx4_t = io_pool.tile([P, NGRP], i32, tag="idx4")
                nc.sync.dma_start(
                    out=idx4_t[:],
                    in_=idx4[t * P * RSLOT : (t + 1) * P * RSLOT, 0].rearrange(
                        "(p g) -> p g", g=NGRP
                    ),
                )
                idx4_tiles.append(idx4_t)
                slots16 = io_pool.tile([P, DIM], f16, tag="slots16")
                nc.sync.dma_start(out=slots16[:], in_=slots[tok, :])
                slots_tiles.append(slots16)

            # ---- host-precomputed constants (needed ~halfway in) ----
            ident = mpool.tile([P, P], f16)
            nc.sync.dma_start(out=ident[:], in_=ident_c[:])
            E_pack = mpool.tile([RANK, RSLOT * P], f16)
            nc.sync.dma_start(out=E_pack[:], in_=e_c[:])
            M4 = mpool.tile([P, P], f16)  # M4[p, t] = (p//4 == t%32)
            nc.sync.dma_start(out=M4[:], in_=m4_c[:])
            # zero-padded lhsT buffers, (tile, h, g): zero except columns
            # 32g..32g+32 (refilled per tile; zeros persist)
            lhsT_all = mpool.tile([P, N_TILE, RSLOT, NGRP, P], f16)
            nc.scalar.memzero(lhsT_all[:])

            # ---- phase A: down gathers + down projection + lhsT build ----
            for t in range(N_TILE):
                idx_t = idx_tiles[t]
                slots16 = slots_tiles[t]
                # down projection -> ds16 [128, 16] f16. The very first
                # chunk is split small so the DVE stream starts early.
                chunk_plan = [4, 4, 8] if t == 0 else [8, 8]
                ds_a = io_pool.tile([P, RANK // 2], f32, tag="ds_a")  # even ranks (ACT)
                ds_b = io_pool.tile([P, RANK // 2], f32, tag="ds_b")  # odd ranks (DVE)
                r0 = 0
                for nr in chunk_plan:
                    dch = gpool.tile([P, nr, DIM], f16, tag=f"dch{nr}")
                    nc.gpsimd.indirect_dma_start(
                        out=dch[:].rearrange("p r d -> p (r d)"),
                        out_offset=None,
                        in_=down[:],
                        in_offset=bass.IndirectOffsetOnAxis(ap=idx_t[:, :1], axis=0),
                        element_offset=r0 * DIM,
                    )
                    for rl in range(nr):
                        r = r0 + rl
                        if rl % 2 == 0:
                            # DVE 4x product + ACT free-dim sum
                            prod = upool.tile([P, DIM], f16, tag="prod")
                            nc.vector.tensor_tensor(
                                out=prod[:],
                                in0=slots16[:],
                                in1=dch[:, rl, :],
                                op=mult,
                            )
                            nc.scalar.activation(
                                out=scratch[:],
                                in_=prod[:],
                                func=mybir.ActivationFunctionType.Copy,
                                accum_out=ds_a[:, r // 2 : r // 2 + 1],
                            )
                        else:
                            # fused multiply+reduce on DVE
                            nc.vector.scalar_tensor_tensor(
                                out=scratch[:],
                                in0=slots16[:],
                                scalar=1.0,
                                in1=dch[:, rl, :],
                                op0=mult,
                                op1=mult,
                                accum_out=ds_b[:, r // 2 : r // 2 + 1],
                            )
                    r0 += nr

                ds16 = io_pool.tile([P, RANK], f16, tag="ds16")
                nc.vector.tensor_copy(
                    out=ds16[:].rearrange("p (a two) -> p a two", two=2)[:, :, 0],
                    in_=ds_a[:],
                )
                nc.vector.tensor_copy(
                    out=ds16[:].rearrange("p (a two) -> p a two", two=2)[:, :, 1],
                    in_=ds_b[:],
                )
                # build the block-diagonal lhsT family from ds
                dsT_psum = pspool.tile([RANK, P], f16, space="PSUM", tag="dsT")
                nc.tensor.transpose(out=dsT_psum[:], in_=ds16[:], identity=ident[:])
                dsT = io_pool.tile([RANK, P], f16, tag="dsT")
                nc.vector.tensor_copy(out=dsT[:], in_=dsT_psum[:])
                for h in range(RSLOT):
                    rep_psum = pspool.tile([P, P], f32, space="PSUM", tag="rep")
                    nc.tensor.matmul(
                        out=rep_psum[:],
                        lhsT=E_pack[:, h * P : (h + 1) * P],
                        rhs=dsT[:],
                        start=True,
                        stop=True,
                    )
                    for g in range(NGRP):
                        cs = slice(TPG * g, TPG * (g + 1))
                        nc.vector.tensor_tensor(
                            out=lhsT_all[:, t, h, g, cs],
                            in0=rep_psum[:, cs],
                            in1=M4[:, cs],
                            op=mult,
                        )

            # ---- phase B: up gathers all issued up front ----
            upc_tiles = {}
            for t in range(N_TILE):
                for g in range(NGRP):
                    upc = upool.tile([P, RSLOT * DIM], f16, tag="upc")
                    nc.gpsimd.indirect_dma_start(
                        out=upc[:],
                        out_offset=None,
                        in_=up4[:],
                        in_offset=bass.IndirectOffsetOnAxis(
                            ap=idx4_tiles[t][:, g : g + 1], axis=0
                        ),
                    )
                    upc_tiles[t, g] = upc

            # ---- up projection on TensorE ----
            for t in range(N_TILE):
                tok = slice(t * P, (t + 1) * P)
                out_psum = pspool_out.tile([P, DIM], f32, space="PSUM", tag="outp")
                for g in range(NGRP):
                    upc = upc_tiles[t, g]
                    for h in range(RSLOT):
                        for n in range(2):
                            n0, n1 = n * 512, (n + 1) * 512
                            nc.tensor.matmul(
                                out=out_psum[:, n0:n1],
                                lhsT=lhsT_all[:, t, h, g, :],
                                rhs=upc[:, h * DIM + n0 : h * DIM + n1],
                                start=(g == 0 and h == 0),
                                stop=(g == NGRP - 1 and h == RSLOT - 1),
                            )
                out_sb = io_pool.tile([P, DIM], f32, tag="osb")
                nc.scalar.mul(out_sb[:], out_psum[:], SCALE)
                nc.sync.dma_start(out=out[tok, :], in_=out_sb[:])
    nc.compile()
    return nc


def _get_nc():
    if "nc" not in _CACHE:
        _CACHE["nc"] = _build()
    return _CACHE["nc"]


def _prep_in_maps(slots, indices, down_proj_values, up_proj_values):
    slots = np.ascontiguousarray(np.asarray(slots, dtype=np.float32).astype(np.float16))
    indices = np.ascontiguousarray(np.asarray(indices).astype(np.int32))
    down = np.ascontiguousarray(
        np.asarray(down_proj_values, dtype=np.float32)
        .transpose(0, 2, 1)
        .reshape(NE, ROW)
        .astype(np.float16)
    )
    up4 = np.ascontiguousarray(
        np.asarray(up_proj_values, dtype=np.float32)
        .reshape(NE * RSLOT, RSLOT * DIM)
        .astype(np.float16)
    )
    assert slots.shape == (B, K, DIM) and indices.shape == (B, K)
    # idx4[t*512 + g*128 + p] = indices[128*t + 32*g + p//4]*4 + p%4
    # (tile t, group g of 32 tokens; partition p = (j, rp) = (p//4, p%4);
    #  up4 table viewed as [NE*4, 4096]: row idx*4+rp = ranks 4rp..4rp+4)
    p = np.arange(P)
    j, rp = p // RSLOT, p % RSLOT
    t_i = np.arange(N_TILE)[:, None, None]
    g_i = np.arange(NGRP)[None, :, None]
    toks = 128 * t_i + TPG * g_i + j[None, None, :]  # [N_TILE, NGRP, P]
    # host-built constants
    ident_c = np.eye(P, dtype=np.float16)
    e_c = np.zeros((RANK, RSLOT, P), np.float16)  # E_h[q, x] = (q == 4*(x%4)+h)
    for h in range(RSLOT):
        for x in range(P):
            e_c[RSLOT * (x % RSLOT) + h, h, x] = 1.0
    e_c = e_c.transpose(0, 1, 2).reshape(RANK, RSLOT * P)
    m4_c = (np.arange(P)[:, None] // RSLOT == np.arange(P)[None, :] % TPG).astype(
        np.float16
    )
    in_maps = []
    for i in range(N_CORES):
        idx4v = (
            (indices[i][toks] * RSLOT + rp[None, None, :])
            .astype(np.int32)
            .transpose(0, 2, 1)
        )  # [N_TILE, P, NGRP]
        in_maps.append(
            {
                "slots": slots[i],
                "idx": indices[i].reshape(K, 1),
                "idx4": idx4v.reshape(K * RSLOT, 1),
                "down": down,
                "up4": up4,
                "ident_c": ident_c,
                "e_c": e_c,
                "m4_c": m4_c,
            }
        )
    return in_maps


def _run(in_maps, trace=False):
    from concourse.bass_utils import run_bass_kernel_spmd

    nc = _get_nc()
    return run_bass_kernel_spmd(
        nc, in_maps, core_ids=list(range(N_CORES)), trace=trace
    )


def kernel(slots, indices, down_proj_values, up_proj_values):
    in_maps = _prep_in_maps(slots, indices, down_proj_values, up_proj_values)
    res = _run(in_maps)
    out = np.stack([res.results[i]["out"] for i in range(N_CORES)], axis=0)
    return out.astype(np.float32)
